# revision 3
# baseline (speedup 1.0000x reference)
"""Fused causal multi-head self-attention (pre-LayerNorm) on 8 TRN2 cores.

Problem: X[2,2048,1024] -> LN -> QKV (16 heads, dh=64) -> causal softmax
attention -> output projection.

Sharding: core c handles batch b = c//4 and head group g = c%4 (4 heads).
Each core computes LN(X_b), Q/K/V for its 4 heads, causal attention, and a
partial output projection against its 256 rows of Wo.  The host sums the 4
partial outputs per batch (the all-reduce of the row-sharded projection),
adds bo, and transposes.

Host-side algebraic folds: LN affine and the 1/sqrt(dh) score scale are
folded into the projection weights; bo is applied on host.

Device structure (per core):
  - Contraction dims always live on SBUF partitions (x is staged as x^T),
    so no on-device transposes are needed anywhere.
  - LN stats via ones-vector matmuls; the per-token mean/bias corrections
    of Q,K,V are folded into each projection matmul as a rank-2 PSUM
    accumulation against rows_sb = [mu; 1/rstd], leaving one DVE op per
    projection (the rstd scale).
  - Scores are computed transposed St[k,q] so softmax's k-reduction is a
    PE reduction; V carries a trailing ones column so the AV matmul yields
    the softmax denominator for free.  exp() runs without max-subtraction
    (scores bounded ~|17|, safe in f32->bf16).
  - Software-pipelined emission: chunk qc+1's LN/QKV and earlier chunks'
    output projections are emitted as small "filler" task groups
    interleaved into chunk qc's ACT-bound attention inner loop, so the
    in-order PE queue always has ready work.  AV matmuls trail their
    scores by two k-tiles so cross-engine waits are pre-satisfied.
"""

import os
import numpy as np
import ml_dtypes

S = 2048
D = 1024
DH = 64
H_PER_CORE = 4
HD = H_PER_CORE * DH  # 256
NQ = S // 512  # 4 q-chunks of 512
ND = D // 128  # 8 d-tiles
NS = S // 128  # 16 s/k-tiles
EPS = 1e-4

_CACHE = {}
LAST_RESULT = None


def _build_nc():
    import concourse.bass as bass
    import concourse.mybir as mybir
    import concourse.tile as tile
    from concourse import bacc
    from contextlib import ExitStack

    f32 = mybir.dt.float32
    bf16 = mybir.dt.bfloat16
    MULT = mybir.AluOpType.mult
    SUB = mybir.AluOpType.subtract
    ADD = mybir.AluOpType.add
    EXP = mybir.ActivationFunctionType.Exp
    LN = mybir.ActivationFunctionType.Ln

    nc = bacc.Bacc("TRN2", target_bir_lowering=False, debug=False,
                   num_devices=8)

    xt = nc.dram_tensor("xt", [D, S], bf16, kind="ExternalInput").ap()
    wq = nc.dram_tensor("wq", [128, ND, HD], bf16, kind="ExternalInput").ap()
    wk = nc.dram_tensor("wk", [128, ND, HD], bf16, kind="ExternalInput").ap()
    wv = nc.dram_tensor("wv", [128, ND, HD], bf16, kind="ExternalInput").ap()
    wo = nc.dram_tensor("wo", [128, 2, D], bf16, kind="ExternalInput").ap()
    # qkcorr[:, j, p, :]: j=0 -> Q, j=1 -> K; rows [-colsum(W); b_eff]
    qkcorr = nc.dram_tensor("qkcorr", [2, 2, 2, 128], bf16,
                            kind="ExternalInput").ap()
    vcorr = nc.dram_tensor("vcorr", [2, HD], bf16, kind="ExternalInput").ap()
    mask = nc.dram_tensor("mask", [128, 4, 512], bf16,
                          kind="ExternalInput").ap()
    out = nc.dram_tensor("out", [D, S], f32, kind="ExternalOutput").ap()

    with tile.TileContext(nc) as tc, ExitStack() as ctx:
        const = ctx.enter_context(tc.tile_pool(name="const", bufs=1))
        big = ctx.enter_context(tc.tile_pool(name="big", bufs=1))
        tmp = ctx.enter_context(tc.tile_pool(name="tmp", bufs=3))
        apool = ctx.enter_context(tc.tile_pool(name="apool", bufs=10))
        rpool = ctx.enter_context(tc.tile_pool(name="rpool", bufs=4))
        obuf = ctx.enter_context(tc.tile_pool(name="obuf", bufs=4))
        dpool = ctx.enter_context(tc.tile_pool(name="dram", bufs=1,
                                               space="DRAM"))
        drec = ctx.enter_context(tc.tile_pool(name="drec", bufs=4,
                                              space="DRAM"))
        # PSUM: qkv/stats 2, o 2, sc 2, av 2  -> 8 banks
        ps_qkv = ctx.enter_context(
            tc.tile_pool(name="ps_qkv", bufs=2, space="PSUM"))
        ps_o = ctx.enter_context(
            tc.tile_pool(name="ps_o", bufs=2, space="PSUM"))
        ps_sc = ctx.enter_context(
            tc.tile_pool(name="ps_sc", bufs=2, space="PSUM"))
        ps_av = ctx.enter_context(
            tc.tile_pool(name="ps_av", bufs=2, space="PSUM"))

        # ---- constants / weights
        wq_sb = const.tile([128, ND, HD], bf16, tag="wq")
        wk_sb = const.tile([128, ND, HD], bf16, tag="wk")
        wv_sb = const.tile([128, ND, HD], bf16, tag="wv")
        wo_sb = const.tile([128, 2, D], bf16, tag="wo")
        qkcorr_sb = const.tile([2, 2, 2, 128], bf16, tag="qkcorr")
        vcorr_sb = const.tile([2, HD], bf16, tag="vcorr")
        cmask_sb = const.tile([128, 4, 512], bf16, tag="cmask")
        ones_sb = const.tile([128, 1], bf16, tag="ones")
        onef_sb = const.tile([1, 1], f32, tag="onef")
        ones64_sb = const.tile([1, DH], bf16, tag="ones64")
        nc.vector.memset(ones64_sb, 1.0)
        ones128_sb = const.tile([1, 128], bf16, tag="ones128")
        nc.vector.memset(ones128_sb, 1.0)

        xt_sb = big.tile([128, ND, S], bf16, tag="xt")

        # Combined Ln+Exp activation-table set resident from the start;
        # emitted before anything else lands on the ACT sequencer.
        _ldset = mybir.InstLoadActFuncSet(
            name=nc.get_next_instruction_name(), ins=[], outs=[],
            act_func_set_id=6)
        nc.scalar.add_instruction(_ldset)
        # chunk-0 xt columns first, split across queues
        for dt in range(ND):
            eng = nc.sync if dt % 2 == 0 else nc.scalar
            eng.dma_start(xt_sb[:, dt, 0:512],
                          xt[dt * 128:(dt + 1) * 128, 0:512])
        nc.scalar.dma_start(wq_sb, wq)
        nc.scalar.dma_start(wk_sb, wk)
        nc.scalar.dma_start(wv_sb, wv)
        nc.scalar.dma_start(wo_sb, wo)
        nc.sync.dma_start(qkcorr_sb, qkcorr)
        nc.sync.dma_start(vcorr_sb, vcorr)
        nc.sync.dma_start(cmask_sb, mask)
        nc.vector.memset(ones_sb, 1.0)
        nc.vector.memset(onef_sb, 1.0)

        # ---- persistent activations
        qt_sb = big.tile([128, 2, S], bf16, tag="qt")
        kt_sb = big.tile([128, 2, S], bf16, tag="kt")
        v_sb = big.tile([128, NS, H_PER_CORE, DH + 1], bf16, tag="v")
        avt_sb = big.tile([128, 2, S], bf16, tag="avt")
        mu_sb = big.tile([1, S], f32, tag="mu")
        rstd_sb = big.tile([1, S], f32, tag="rstd")
        rows_sb = big.tile([2, S], bf16, tag="rows")   # [mu; 1/rstd]
        rinv_sb = big.tile([1, S], bf16, tag="rinv")   # staging (partition 0)
        rstdb_sb = big.tile([128, S], f32, tag="rstdb")
        rstd_dr = dpool.tile([1, S], f32, tag="rstdd")
        rcol_sb = big.tile([128, NS], f32, tag="rcol")

        # V's trailing all-ones column (softmax denominator trick)
        nc.vector.memset(v_sb[:, :, :, DH:DH + 1], 1.0)

        # ================= task groups =================

        def make_chunk_tasks(qc):
            """Closures emitting LN + QKV for chunk qc, in dependency
            order.  Each closure is one small instruction group."""
            qs = slice(qc * 512, (qc + 1) * 512)
            tasks = []
            state = {}

            if qc > 0:
                def t_xt():
                    for dt in range(ND):
                        eng = nc.sync if dt % 2 == 0 else nc.scalar
                        eng.dma_start(xt_sb[:, dt, qs],
                                      xt[dt * 128:(dt + 1) * 128, qs])
                tasks.append(t_xt)

            def t_mu():
                mu_ps = ps_qkv.tile([1, 512], f32, tag="qkv",
                                    name=f"mu_ps{qc}")
                for dt in range(ND):
                    nc.tensor.matmul(mu_ps, ones_sb, xt_sb[:, dt, qs],
                                     start=(dt == 0), stop=(dt == ND - 1))
                nc.vector.tensor_scalar_mul(mu_sb[0:1, qs], mu_ps, 1.0 / D)
                nc.vector.tensor_scalar_mul(rows_sb[0:1, qs], mu_ps, 1.0 / D)
            sq_eng = nc.vector if qc == 0 else nc.gpsimd

            def t_sq_all():
                # all 8 squares issue up-front so the (slow) Pool engine
                # has a head start on the ss matmuls that consume them
                for dt in range(ND):
                    xsl = xt_sb[:, dt, qs]
                    sq = tmp.tile([128, 512], bf16, tag="sq",
                                  name=f"sq{qc}_{dt}", bufs=8)
                    state[f"sq{dt}"] = sq
                    sq_eng.tensor_tensor(out=sq, in0=xsl, in1=xsl, op=MULT)
            tasks.append(t_sq_all)
            tasks.append(t_mu)

            def t_ss(half):
                def f():
                    if half == 0:
                        state["ss_ps"] = ps_qkv.tile(
                            [1, 512], f32, tag="qkv", name=f"ss_ps{qc}")
                    for dt in range(4 * half, 4 * half + 4):
                        nc.tensor.matmul(state["ss_ps"], ones_sb,
                                         state[f"sq{dt}"],
                                         start=(dt == 0),
                                         stop=(dt == ND - 1))
                return f
            tasks.append(t_ss(0))
            tasks.append(t_ss(1))

            def t_lntail():
                ss_ps = state["ss_ps"]
                mu_c = mu_sb[0:1, qs]
                msq = tmp.tile([1, 512], f32, tag="msq", name=f"msq{qc}")
                nc.vector.tensor_tensor(out=msq, in0=mu_c, in1=mu_c, op=MULT)
                vpe = tmp.tile([1, 512], f32, tag="vpe", name=f"vpe{qc}")
                nc.vector.tensor_scalar(out=vpe, in0=ss_ps,
                                        scalar1=1.0 / D, scalar2=EPS,
                                        op0=MULT, op1=ADD)
                var = tmp.tile([1, 512], f32, tag="var", name=f"var{qc}")
                nc.vector.tensor_tensor(out=var, in0=vpe, in1=msq, op=SUB)
                lnv = tmp.tile([1, 512], f32, tag="lnv", name=f"lnv{qc}")
                nc.scalar.activation(lnv, var, LN)
                nc.scalar.activation(rstd_sb[0:1, qs], lnv, EXP, scale=-0.5)
                # engines can't write at partition offset 1: stage the
                # 1/rstd row at partition 0 and DMA it into rows_sb row 1
                nc.scalar.activation(rinv_sb[0:1, qs], lnv, EXP, scale=0.5)
                nc.scalar.dma_start(rows_sb[1:2, qs], rinv_sb[0:1, qs])
                nc.sync.dma_start(rstd_dr[0:1, qs], rstd_sb[0:1, qs])
                nc.sync.dma_start(rstdb_sb[:, qs],
                                  rstd_dr[0:1, qs].partition_broadcast(128))
            tasks.append(t_lntail)

            def qk_main(p, j, w_sb, pool, tag):
                def f():
                    hp = slice(p * 128, (p + 1) * 128)
                    state[f"qk{p}{j}"] = ps = pool.tile(
                        [128, 512], f32, tag=tag, name=f"qk_ps{qc}_{p}_{j}")
                    for dt in range(ND):
                        nc.tensor.matmul(ps, w_sb[:, dt, hp],
                                         xt_sb[:, dt, qs],
                                         start=(dt == 0), stop=False)
                return f

            def qk_fin(p, j, dst):
                def f():
                    ps = state[f"qk{p}{j}"]
                    nc.tensor.matmul(ps, qkcorr_sb[:, j, p, :],
                                     rows_sb[:, qs], start=False, stop=True)
                    nc.vector.tensor_tensor(out=dst[:, p, qs], in0=ps,
                                            in1=rstdb_sb[:, qs], op=MULT)
                return f

            def v_main(i, pool, tag):
                def f():
                    st = 4 * qc + i
                    ss_ = slice(st * 128, (st + 1) * 128)
                    state[f"v{i}"] = v_ps = pool.tile(
                        [128, HD], f32, tag=tag, name=f"v_ps{qc}_{i}")
                    for dt in range(ND):
                        nc.tensor.matmul(v_ps, xt_sb[:, dt, ss_],
                                         wv_sb[:, dt, :],
                                         start=(dt == 0), stop=False)
                return f

            def v_fin(i):
                def f():
                    st = 4 * qc + i
                    ss_ = slice(st * 128, (st + 1) * 128)
                    v_ps = state[f"v{i}"]
                    nc.tensor.matmul(v_ps, rows_sb[:, ss_], vcorr_sb,
                                     start=False, stop=True)
                    nc.vector.tensor_scalar_mul(
                        v_sb[:, st, :, 0:DH],
                        v_ps.rearrange("p (h d) -> p h d", h=H_PER_CORE),
                        rcol_sb[:, st:st + 1])
                return f

            combos = ((0, 0), (0, 1), (1, 0), (1, 1))
            if qc == 0:
                # startup: sc/av/o pools are idle — borrow them so all QKV
                # mains can be in flight before the rows chain resolves
                qpools = [(ps_qkv, "qkv"), (ps_qkv, "qkv"),
                          (ps_sc, "sc"), (ps_sc, "sc")]
                vpools = [(ps_av, "av"), (ps_av, "av"),
                          (ps_o, "o"), (ps_o, "o")]
                for i, (p, j) in enumerate(combos):
                    w_sb = wq_sb if j == 0 else wk_sb
                    tasks.append(qk_main(p, j, w_sb, *qpools[i]))
                for i in range(4):
                    tasks.append(v_main(i, *vpools[i]))
            else:
                seq = []
                for p, j in combos:
                    w_sb = wq_sb if j == 0 else wk_sb
                    seq.append((qk_main(p, j, w_sb, ps_qkv, "qkv"),
                                qk_fin(p, j, qt_sb if j == 0 else kt_sb)))
                tasks.append(seq[0][0])          # mainA
                tasks.append(seq[1][0])          # mainB
                tasks.append(seq[0][1])          # finA
                tasks.append(seq[2][0])          # mainC
                tasks.append(seq[1][1])          # finB
                tasks.append(seq[3][0])          # mainD
                tasks.append(seq[2][1])          # finC
                tasks.append(seq[3][1])          # finD

            def t_rcol():
                colps = ps_qkv.tile([128, 4], f32, tag="qkv",
                                    name=f"colps{qc}")
                for i in range(4):
                    st_ = slice((4 * qc + i) * 128, (4 * qc + i + 1) * 128)
                    nc.tensor.matmul(colps[:, i:i + 1], rstd_sb[0:1, st_],
                                     onef_sb, start=True, stop=True)
                cts = slice(4 * qc, 4 * qc + 4)
                nc.vector.tensor_copy(rcol_sb[:, cts], colps[:, 0:4])
            tasks.append(t_rcol)

            if qc == 0:
                for i, (p, j) in enumerate(combos):
                    tasks.append(qk_fin(p, j, qt_sb if j == 0 else kt_sb))
                for i in range(4):
                    tasks.append(v_fin(i))
            else:
                for i in range(4):
                    def t_v(i=i):
                        v_main(i, ps_qkv, "qkv")()
                        v_fin(i)()
                    tasks.append(t_v)
            return tasks

        def outproj_tasks(qc):
            qs = slice(qc * 512, (qc + 1) * 512)
            tasks = []

            def t_o(ot, pool, tag="o"):
                def f():
                    o_ps = pool.tile([128, 512], f32, tag=tag,
                                     name=f"o_ps{qc}_{ot}")
                    osl = slice(ot * 128, (ot + 1) * 128)
                    for p in range(2):
                        nc.tensor.matmul(o_ps, wo_sb[:, p, osl],
                                         avt_sb[:, p, qs],
                                         start=(p == 0), stop=(p == 1))
                    o_sb = obuf.tile([128, 512], f32, tag="ob",
                                     name=f"o_sb{qc}_{ot}")
                    nc.vector.tensor_copy(o_sb, o_ps)
                    eng = nc.sync if ot % 2 == 0 else nc.scalar
                    eng.dma_start(out[osl, qs], o_sb)
                return f
            for ot in range(ND):
                if qc == NQ - 1:
                    pool = [ps_sc, ps_o][ot % 2]
                    tag = ["sc", "o"][ot % 2]
                else:
                    pool, tag = ps_o, "o"
                tasks.append(t_o(ot, pool, tag))
            return tasks

        def attention(qt, fillers):
            """Causal attention for q-chunk qt, interleaving filler tasks."""
            qs = slice(qt * 512, (qt + 1) * 512)
            nkt = 4 * qt + 4
            slots = 2 * nkt
            fi = [0]

            def pop_fill(slot):
                remaining = len(fillers) - fi[0]
                slots_left = slots - slot
                n = -(-remaining // slots_left) if slots_left > 0 else remaining
                for _ in range(n):
                    if fi[0] < len(fillers):
                        fillers[fi[0]]()
                        fi[0] += 1

            slot = 0
            for p in range(2):
                av0 = ps_av.tile([DH + 1, 512], f32, tag="av",
                                 name=f"av0_{qt}_{p}")
                av1 = ps_av.tile([DH + 1, 512], f32, tag="av",
                                 name=f"av1_{qt}_{p}")
                pends = []  # AV matmuls delayed by two k-tiles
                for kt in range(nkt):
                    ks = slice(kt * 128, (kt + 1) * 128)
                    jj = kt - 4 * qt
                    vls = slice(max(0, jj) * 128, 512)
                    qv = qt_sb[:, p, qt * 512 + vls.start:(qt + 1) * 512]
                    sc0 = ps_sc.tile([128, 512], f32, tag="sc",
                                     name=f"sc0_{qt}_{p}_{kt}")
                    # last chunk: qkv banks are idle (no next chunk) —
                    # borrow as a third score slot to deepen the exp pipe
                    if qt == NQ - 1 and kt % 2 == 1:
                        sc1 = ps_qkv.tile([128, 512], f32, tag="qkv",
                                          name=f"sc1_{qt}_{p}_{kt}")
                    else:
                        sc1 = ps_sc.tile([128, 512], f32, tag="sc",
                                         name=f"sc1_{qt}_{p}_{kt}")
                    nc.tensor.matmul(sc0[:, vls], kt_sb[0:64, p, ks],
                                     qv[0:64, :], start=True, stop=True)
                    nc.tensor.matmul(sc1[:, vls], kt_sb[64:128, p, ks],
                                     qv[64:128, :], start=True, stop=True)
                    a0 = apool.tile([128, 512], bf16, tag="a",
                                    name=f"a0_{qt}_{p}_{kt}")
                    a1 = apool.tile([128, 512], bf16, tag="a",
                                    name=f"a1_{qt}_{p}_{kt}")
                    nc.scalar.activation(a0[:, vls], sc0[:, vls], EXP)
                    nc.scalar.activation(a1[:, vls], sc1[:, vls], EXP)
                    if jj >= 0:
                        dsl = slice(jj * 128, (jj + 1) * 128)
                        msl = cmask_sb[:, jj, dsl]
                        nc.vector.tensor_tensor(out=a0[:, dsl],
                                                in0=a0[:, dsl],
                                                in1=msl, op=MULT)
                        nc.vector.tensor_tensor(out=a1[:, dsl],
                                                in0=a1[:, dsl],
                                                in1=msl, op=MULT)
                    if len(pends) >= 2:
                        pends.pop(0)()
                    pk, pa0, pa1, pvls = kt, a0, a1, vls

                    def mk_pend(pk, pa0, pa1, pvls):
                        def f():
                            nc.tensor.matmul(av0[:, pvls],
                                             v_sb[:, pk, 2 * p, :],
                                             pa0[:, pvls],
                                             start=(pk == 0),
                                             stop=(pk == nkt - 1))
                            nc.tensor.matmul(av1[:, pvls],
                                             v_sb[:, pk, 2 * p + 1, :],
                                             pa1[:, pvls],
                                             start=(pk == 0),
                                             stop=(pk == nkt - 1))
                        return f
                    pends.append(mk_pend(pk, pa0, pa1, pvls))
                    pop_fill(slot)
                    slot += 1
                for pd in pends:
                    pd()
                # normalize by softmax denominator (row DH)
                for j, av_ps in ((0, av0), (1, av1)):
                    hrow = slice(64 * j, 64 * j + 64)
                    if qt == NQ - 1:
                        # latency-critical tail: broadcast 1/denom via a
                        # K=1 matmul into a then-idle qkv PSUM bank plus a
                        # Pool copy, instead of the DRAM DMA round-trip
                        recip_bf = rpool.tile([1, 512], bf16, tag="recip",
                                              name=f"recipb{qt}_{p}_{j}")
                        with nc.allow_low_precision(
                                reason="bf16 1/denom, 0.4% rel"):
                            nc.vector.reciprocal(recip_bf,
                                                 av_ps[DH:DH + 1, :])
                        recb_ps = ps_qkv.tile([DH, 512], f32, tag="qkv",
                                              name=f"recbp{qt}_{p}_{j}")
                        nc.tensor.matmul(recb_ps, ones64_sb, recip_bf,
                                         start=True, stop=True)
                        recb = rpool.tile([DH, 512], f32, tag="recb",
                                          name=f"recb{qt}_{p}_{j}")
                        nc.vector.tensor_copy(recb, recb_ps)
                    else:
                        recip = rpool.tile([1, 512], f32, tag="recip",
                                           name=f"recip{qt}_{p}_{j}")
                        nc.vector.reciprocal(recip, av_ps[DH:DH + 1, :])
                        rec_dr = drec.tile([1, 512], f32, tag="recd",
                                           name=f"recd{qt}_{p}_{j}")
                        nc.sync.dma_start(rec_dr, recip)
                        recb = rpool.tile([DH, 512], f32, tag="recb",
                                          name=f"recb{qt}_{p}_{j}")
                        nc.sync.dma_start(recb,
                                          rec_dr.partition_broadcast(DH))
                    nc.vector.tensor_tensor(out=avt_sb[hrow, p, qs],
                                            in0=av_ps[0:DH, :],
                                            in1=recb, op=MULT)
            # drain any remaining fillers
            while fi[0] < len(fillers):
                fillers[fi[0]]()
                fi[0] += 1

        # ================= schedule =================
        # outproj(qc) only needs avt(qc): defer each to a later, less
        # loaded attention stretch to balance PE demand per chunk.
        for t in make_chunk_tasks(0):
            t()
        for qc in range(NQ):
            fillers = []
            if qc == 3:
                fillers += (outproj_tasks(0) + outproj_tasks(1)
                            + outproj_tasks(2))
            if qc + 1 < NQ:
                fillers += make_chunk_tasks(qc + 1)
            attention(qc, fillers)
        for t in outproj_tasks(NQ - 1):
            t()

    nc.compile()
    return nc


def _prep_in_maps(inputs):
    bf = ml_dtypes.bfloat16
    X = np.asarray(inputs["X"], np.float32)
    ln_w = np.asarray(inputs["ln_w"], np.float32)
    ln_b = np.asarray(inputs["ln_b"], np.float32)
    Wq = np.asarray(inputs["Wq"], np.float32)
    Wk = np.asarray(inputs["Wk"], np.float32)
    Wv = np.asarray(inputs["Wv"], np.float32)
    Wo = np.asarray(inputs["Wo"], np.float32)
    bq = np.asarray(inputs["bq"], np.float32)
    bk = np.asarray(inputs["bk"], np.float32)
    bv = np.asarray(inputs["bv"], np.float32)

    scale = 1.0 / np.sqrt(DH).astype(np.float32)
    Wq_eff = ln_w[:, None] * Wq * scale
    bq_eff = (ln_b @ Wq + bq) * scale
    Wk_eff = ln_w[:, None] * Wk
    bk_eff = ln_b @ Wk + bk
    Wv_eff = ln_w[:, None] * Wv
    bv_eff = ln_b @ Wv + bv

    # cmask[i, jj, q] = 1 if q >= 128*jj + i else 0
    ii = np.arange(128)[:, None, None]
    jjj = np.arange(4)[None, :, None]
    qq = np.arange(512)[None, None, :]
    mask = (qq >= 128 * jjj + ii).astype(np.float32).astype(bf)

    # column sums of the bf16-rounded effective weights
    csq_full = Wq_eff.astype(bf).astype(np.float32).sum(axis=0)
    csk_full = Wk_eff.astype(bf).astype(np.float32).sum(axis=0)
    csv_full = Wv_eff.astype(bf).astype(np.float32).sum(axis=0)

    in_maps = []
    for c in range(8):
        b, g = c // 4, c % 4
        hs = slice(g * HD, (g + 1) * HD)
        qkcorr = np.zeros((2, 2, 2, 128), np.float32)
        for j, (cs, bias) in enumerate(((csq_full, bq_eff),
                                        (csk_full, bk_eff))):
            for p in range(2):
                psl = slice(g * HD + p * 128, g * HD + (p + 1) * 128)
                qkcorr[0, j, p, :] = -cs[psl]
                qkcorr[1, j, p, :] = bias[psl]
        vcorr = np.stack([-csv_full[hs], bv_eff[hs]])
        in_maps.append({
            "xt": np.ascontiguousarray(X[b].T).astype(bf),
            "wq": np.ascontiguousarray(
                Wq_eff[:, hs].reshape(ND, 128, HD).transpose(1, 0, 2)
            ).astype(bf),
            "wk": np.ascontiguousarray(
                Wk_eff[:, hs].reshape(ND, 128, HD).transpose(1, 0, 2)
            ).astype(bf),
            "wv": np.ascontiguousarray(
                Wv_eff[:, hs].reshape(ND, 128, HD).transpose(1, 0, 2)
            ).astype(bf),
            "wo": np.ascontiguousarray(
                Wo[hs, :].reshape(2, 128, D).transpose(1, 0, 2)
            ).astype(bf),
            "qkcorr": qkcorr.astype(bf),
            "vcorr": vcorr.astype(bf),
            "mask": mask,
        })
    return in_maps


def kernel(**inputs) -> np.ndarray:
    global LAST_RESULT
    from concourse.bass_utils import run_bass_kernel_spmd

    if "nc" not in _CACHE:
        _CACHE["nc"] = _build_nc()
    nc = _CACHE["nc"]

    in_maps = _prep_in_maps(inputs)
    res = run_bass_kernel_spmd(
        nc, in_maps, core_ids=list(range(8)),
        trace=bool(int(os.environ.get("KERNEL_TRACE", "0"))),
    )
    LAST_RESULT = res
    outs = [r["out"] for r in res.results]
    bo = np.asarray(inputs["bo"], np.float32)
    full = np.stack([
        (outs[0] + outs[1] + outs[2] + outs[3]).T,
        (outs[4] + outs[5] + outs[6] + outs[7]).T,
    ]).astype(np.float32) + bo
    return full
